# revision 16
# baseline (speedup 1.0000x reference)
"""CenterLoss update kernel for Trainium2, 8-core SPMD.

Reference computation (N=16384 samples, C=10000 classes, D=128 dims):
    embeded_labels = labels @ center          # [N,D] gather via one-hot
    diff = embeded_labels - embeded_preds
    grad = (labels.T @ diff) / (counts + 1)   # counts = labels.T @ ones
    out  = center - 0.5 * grad

Because each row of ``labels`` is one-hot, ``labels.T @ labels == diag(counts)``,
so the whole thing collapses to a single pass over ``labels``:

    S      = labels.T @ embeded_preds         # [C,D] per-class sum of preds
    counts = column sums of labels            # [C]
    out    = beta * center + gamma * S
             beta  = 1 - 0.5*counts/(counts+1)
             gamma = 0.5/(counts+1)

Sharding: by CLASS, not batch.  Core i owns classes [i*1250, (i+1)*1250): it
streams labels[:, shard] (the same 82 MB/core the batch split would read),
additionally reads all of preds (only 8.4 MB), and produces its [1250, 128]
slice of the output with NO inter-core communication at all -- the batch-split
variant needs a serial ReduceScatter of the [C,D] partials which measured
~85us of un-overlappable tail.

The 655MB ``labels`` tensor is the whole game (target ~358 GB/s/core HBM):
  * it streams through the PE exactly once as the *moving* matmul operand in
    a single fp32r pass (fp32r moving rows cost 1 PE cycle at width>=256,
    same as bf16; the rounding error lands ~2e-4 rel, far inside the 2e-2
    gate);
  * each SBUF partition line packs FOUR consecutive DRAM rows, so every DMA
    descriptor is a 20 KB contiguous read (a [128 rows-on-partitions] layout
    gives 5 KB descriptors, which caps each of the 16 SDMA engines at
    ~21 GB/s -- measured); the four row-parities just become four stationary
    preds tiles per k-group;
  * the class shard is host-padded 1250 -> 1280 so all matmul chunks are
    512/512/256 wide -- fp32r moving runs 4 cycles/row below width 256;
  * label DMAs alternate between the two HWDGE rings (sync + scalar) so
    ring issue costs overlap across consecutive tiles.
Per-partition partial counts accumulate on the DVE and are reduced by one
final PE pass against a ones vector; the [d, cs] S.T accumulator stays in
PSUM for the whole stream and is transposed on-chip for the tail update.
"""

import numpy as np

N, C, D = 16384, 10000, 128
NCORES = 8
CS = C // NCORES   # 1250 classes per core
CSP = 1280         # padded shard width (host pads with zero columns)
LR = 0.5
P = 128
R = 4              # DRAM rows interleaved per SBUF partition line
GR = R * P         # rows per k-group (512)
PJ = 16            # preds load chunks (at full size)


def _chunks(width, step=512):
    out = []
    c0 = 0
    while c0 < width:
        out.append((c0, min(step, width - c0)))
        c0 += step
    return out


def build_program(n=N, cs=CS, csp=CSP, d=D):
    """Build the SPMD Bass program (identical on every core)."""
    import concourse.bacc as bacc
    import concourse.mybir as mybir
    import concourse.tile as tile
    from concourse.masks import make_identity

    f32 = mybir.dt.float32
    f32r = mybir.dt.float32r
    mult = mybir.AluOpType.mult
    add = mybir.AluOpType.add

    assert n % GR == 0
    ng = n // GR             # k-groups (32)
    pj = min(PJ, ng)         # preds chunks
    pgc = ng // pj           # k-groups per preds chunk
    assert ng % pj == 0
    nt3 = (cs + P - 1) // P  # class tiles for the final update (10)

    nc = bacc.Bacc(
        "TRN2",
        target_bir_lowering=False,
        debug=False,
        num_devices=NCORES,
    )

    # preds is declared float32r exactly like labels: the HWDGE DMA rounds
    # fp32 -> fp32r in flight (trace shows in_dtype=fp32 out_dtype=fp32r),
    # which satisfies the BIR verifier's "matmul inputs must be rounded"
    # rule with zero extra compute.
    preds = nc.dram_tensor("preds", [n, d], f32r, kind="ExternalInput").ap()
    # labels are one-hot 0/1: declaring them float32r (same bits, trivially
    # rounded) lets plain HWDGE DMAs feed fp32r matmuls at full speed.
    labels = nc.dram_tensor("labels", [n, csp], f32r, kind="ExternalInput").ap()
    center = nc.dram_tensor("center", [cs, d], f32, kind="ExternalInput").ap()
    out = nc.dram_tensor("out", [cs, d], f32, kind="ExternalOutput").ap()

    with tile.TileContext(nc) as tc:
        with tc.tile_pool(name="const", bufs=1) as const_pool:
            identity = const_pool.tile([P, P], f32, name="identity")
            make_identity(nc, identity[:])
            ones_f32 = const_pool.tile([P, 1], f32, name="ones_f32")
            nc.vector.memset(ones_f32[:], 1.0)
            ones_col = const_pool.tile([P, 1], f32r, name="ones_col")
            nc.vector.tensor_copy(out=ones_col[:], in_=ones_f32[:])

            # all of preds as ng*R stationary [K=128, M=d] tiles (one per
            # k-group x row-parity); rounded to f32r by the DMA itself, so
            # a single fp32r matmul pass suffices with no cast pass.
            preds_r = const_pool.tile([P, ng * R * d], f32r, name="preds_r")

            # per-partition partial counts, accumulated on DVE.  (GpSimd
            # looks like the obvious offload engine but measures ~4us per
            # [128,1280] add AND degrades concurrent DVE ops ~2.7x via
            # SBUF-port interference -- keep it idle.)  Two parities are
            # added per instruction ([128, 2*csp]); the column blocks are
            # summed against each other by the final accumulating PE
            # reduce, so no combine pass is needed.
            # f32r: counts are small integers (exact), and it makes the
            # final ones-matmul reduce run at fp32r speed (1 cyc/row, not 4)
            cnt_sb = const_pool.tile([P, 2 * csp], f32r, name="cnt_sb")
            # S.T evacuated from PSUM, and the reduced counts row
            st_sb = const_pool.tile([P, cs], f32, name="st_sb")
            cnt_row = const_pool.tile([1, csp], f32, name="cnt_row")
            # center shard, as nt3 [class, d] tiles (class on partitions)
            ctr_sb = const_pool.tile([P, nt3 * d], f32, name="ctr_sb")

            # ---------------- phase 1: stream labels ----------------
            with (
                tc.tile_pool(name="lab", bufs=5) as lab_pool,
                tc.tile_pool(name="psum1", bufs=1, space="PSUM") as psum1,
            ):
                st_psum = psum1.tile([d, csp], f32, name="st_psum", tag="st",
                                     space="PSUM")
                cnt_psum = psum1.tile([P, 3 * 512], f32, name="cnt_psum",
                                      tag="cnt", space="PSUM")

                def load_preds_chunk(j, eng):
                    # one preds chunk (pgc k-groups): partition p takes rows
                    # 4p..4p+3 of each k-group (2 KB contiguous descriptors).
                    # Emission order IS dataflow order in Tile, so chunk j
                    # must be emitted before the first matmul that reads it,
                    # with enough lead: small descriptors get starved by the
                    # 20 KB label packets in the engines' per-packet
                    # round-robin (~14us for a 1 MB chunk under load).
                    c0, c1 = j * pgc * R * d, (j + 1) * pgc * R * d
                    eng.dma_start(
                        out=preds_r[:, c0:c1].rearrange(
                            "p (k r d) -> p k r d", k=pgc, r=R),
                        in_=preds[j * pgc * GR:(j + 1) * pgc * GR, :]
                            .rearrange("(k p r) d -> p k r d", p=P, r=R),
                    )

                # chunk 0 goes at the head of the sync ring, before lab 0:
                # it completes in ~2us so the PE starts ~25us earlier than
                # if it queued behind label traffic.
                load_preds_chunk(0, nc.sync)
                for g in range(ng):
                    lab_g = lab_pool.tile([P, R * csp], f32r, name=f"lab_{g}",
                                          tag="lab")
                    # alternate the two HWDGE rings so ring issue costs and
                    # completion latencies overlap across consecutive tiles;
                    # partition p reads DRAM rows g*512+4p..+3 = one 20 KB
                    # contiguous descriptor.
                    eng = nc.sync if g % 2 == 0 else nc.scalar
                    if g == 0:
                        # two half-tile DMAs so the first matmuls can start
                        # as soon as parities 0-1 land
                        src4 = labels[0:GR, :].rearrange("(p x) c -> p x c",
                                                         x=R)
                        for hh in range(2):
                            eng.dma_start(
                                out=lab_g[:, hh * 2 * csp:(hh + 1) * 2 * csp]
                                    .rearrange("p (r c) -> p r c", r=2),
                                in_=src4[:, 2 * hh:2 * hh + 2, :],
                            )
                    else:
                        eng.dma_start(
                            out=lab_g[:],
                            in_=labels[g * GR:(g + 1) * GR, :]
                                .rearrange("(p r) c -> p (r c)", r=R),
                        )
                    for j in range(1, pj):
                        # interleave the remaining preds chunks into the
                        # scalar ring, each >=4 groups before its first
                        # consumer (chunk j feeds k-groups g >= j*pgc)
                        if g == max(0, min(2 * j - 4, j * pgc - 1)):
                            load_preds_chunk(j, nc.scalar)
                    if g == min(5, ng - 1):
                        # center shard, needed only for the tail update
                        for tt in range(nt3):
                            w = min(P, cs - tt * P)
                            nc.scalar.dma_start(
                                out=ctr_sb[0:w, tt * d:tt * d + d],
                                in_=center[tt * P:tt * P + w, :],
                            )
                    for q in range(R):
                        stat = preds_r[:, (g * R + q) * d:(g * R + q + 1) * d]
                        for c0, w in _chunks(csp):
                            nc.tensor.matmul(
                                out=st_psum[:, c0:c0 + w],
                                lhsT=stat,
                                rhs=lab_g[:, q * csp + c0:q * csp + c0 + w],
                                start=(g == 0 and q == 0),
                                stop=(g == ng - 1 and q == R - 1),
                            )
                    if g < ng - 1:
                        for h in range(2):
                            if g == 0 and h == 0:
                                nc.vector.tensor_copy(
                                    out=cnt_sb[:],
                                    in_=lab_g[:, 0:2 * csp].bitcast(f32),
                                )
                            else:
                                nc.vector.tensor_add(
                                    out=cnt_sb[:], in0=cnt_sb[:],
                                    in1=lab_g[:, h * 2 * csp:
                                              (h + 1) * 2 * csp].bitcast(f32),
                                )
                    else:
                        # last group: column block b of cnt_sb accumulates
                        # parities {b, b+2}, so finish each block with two
                        # narrow adds and reduce it while the DVE works on
                        # the other block
                        for b in range(2):
                            for q in (b, b + 2):
                                nc.vector.tensor_add(
                                    out=cnt_sb[:, b * csp:(b + 1) * csp],
                                    in0=cnt_sb[:, b * csp:(b + 1) * csp],
                                    in1=lab_g[:, q * csp:
                                              (q + 1) * csp].bitcast(f32),
                                )
                            for c0, w in _chunks(csp):
                                nc.tensor.matmul(
                                    out=cnt_psum[0:1, c0:c0 + w],
                                    lhsT=ones_col[:],
                                    rhs=cnt_sb[:, b * csp + c0:
                                               b * csp + c0 + w],
                                    start=(b == 0),
                                    stop=(b == 1),
                                )


                # ACT evacuates S.T as soon as the accumulation group stops
                # (emitted before the counts reduce so its semaphore wait
                # does not include those matmuls)
                nc.scalar.copy(out=st_sb[:], in_=st_psum[:, 0:cs])

                nc.scalar.copy(out=cnt_row[:], in_=cnt_psum[0:1, 0:csp])

                # ---------------- tail: update this core's shard ----------
                # counts column per class tile, transposed into the spare
                # columns of the cnt PSUM bank; padded classes have count 0
                # so every lane stays finite
                for tt in range(nt3):
                    nc.tensor.transpose(
                        out=cnt_psum[0:P, 1300 + tt:1301 + tt],
                        in_=cnt_row[0:1, tt * P:(tt + 1) * P],
                        identity=identity[0:1, 0:1],
                    )
                cnt_col = cnt_psum[:, 1300:1300 + nt3]

                with tc.tile_pool(name="p3", bufs=1) as p3:
                    # per-class scalars for all nt3 tiles in one [P, nt3]
                    # batch: den = counts+1, gam = 0.5/den,
                    # bet = 1 - 0.5*counts/den
                    den = p3.tile([P, nt3], f32, name="den")
                    nc.vector.tensor_scalar_add(
                        out=den[:], in0=cnt_col, scalar1=1.0
                    )
                    rec = p3.tile([P, nt3], f32, name="rec")
                    nc.vector.reciprocal(out=rec[:], in_=den[:])
                    gam = p3.tile([P, nt3], f32, name="gam")
                    nc.vector.tensor_scalar_mul(
                        out=gam[:], in0=rec[:], scalar1=0.5
                    )
                    bet = p3.tile([P, nt3], f32, name="bet")
                    nc.vector.tensor_tensor(
                        out=bet[:], in0=cnt_col, in1=rec[:], op=mult
                    )
                    nc.vector.tensor_scalar(
                        out=bet[:], in0=bet[:],
                        scalar1=-0.5, scalar2=1.0, op0=mult, op1=add,
                    )
                    # all o1 = beta*center products up front so the ou
                    # chain never stalls the DVE FIFO waiting on transposes
                    o1 = p3.tile([P, nt3 * d], f32, name="o1")
                    for tt in range(nt3):
                        w = min(P, cs - tt * P)
                        nc.vector.tensor_scalar_mul(
                            out=o1[0:w, tt * d:tt * d + d],
                            in0=ctr_sb[0:w, tt * d:tt * d + d],
                            scalar1=bet[0:w, tt:tt + 1],
                        )
                    for tt in range(nt3):
                        w = min(P, cs - tt * P)
                        trp = psum1.tile([P, d], f32, name=f"trp_{tt}",
                                         tag="trp", bufs=2, space="PSUM")
                        nc.tensor.transpose(
                            out=trp[0:w, 0:d],
                            in_=st_sb[:, tt * P:tt * P + w],
                            identity=identity[:, 0:d],
                        )
                        ou = p3.tile([P, d], f32, name=f"ou_{tt}", tag="ou",
                                     bufs=2)
                        nc.vector.scalar_tensor_tensor(
                            out=ou[0:w, :], in0=trp[0:w, 0:d],
                            scalar=gam[0:w, tt:tt + 1],
                            in1=o1[0:w, tt * d:tt * d + d], op0=mult, op1=add,
                        )
                        eng3 = nc.sync if tt % 2 == 0 else nc.scalar
                        eng3.dma_start(
                            out=out[tt * P:tt * P + w, :], in_=ou[0:w, 0:d]
                        )

    nc.compile()
    return nc


_PROGRAM = None
LAST_RESULTS = None  # BassKernelResults from the most recent run (for test.py)


def _get_program():
    global _PROGRAM
    if _PROGRAM is None:
        _PROGRAM = build_program()
    return _PROGRAM


def kernel(embeded_preds, labels, center):
    from concourse.bass_utils import run_bass_kernel_spmd

    global LAST_RESULTS
    preds = np.ascontiguousarray(np.asarray(embeded_preds, dtype=np.float32))
    lab = np.asarray(labels, dtype=np.float32)
    ctr = np.ascontiguousarray(np.asarray(center, dtype=np.float32))
    assert preds.shape == (N, D) and lab.shape == (N, C) and ctr.shape == (C, D)

    nc = _get_program()
    in_maps = []
    for i in range(NCORES):
        shard = np.zeros((N, CSP), np.float32)
        shard[:, :CS] = lab[:, i * CS:(i + 1) * CS]
        in_maps.append({
            "preds": preds,
            "labels": shard,
            "center": np.ascontiguousarray(ctr[i * CS:(i + 1) * CS]),
        })
    res = run_bass_kernel_spmd(nc, in_maps, core_ids=list(range(NCORES)))
    LAST_RESULTS = res
    return np.concatenate([res.results[i]["out"] for i in range(NCORES)], axis=0)


# revision 17
# speedup vs baseline: 1.1440x; 1.1440x over previous
"""CenterLoss update kernel for Trainium2, 8-core SPMD.

Reference computation (N=16384 samples, C=10000 classes, D=128 dims):
    embeded_labels = labels @ center          # [N,D] gather via one-hot
    diff = embeded_labels - embeded_preds
    grad = (labels.T @ diff) / (counts + 1)   # counts = labels.T @ ones
    out  = center - 0.5 * grad

Because each row of ``labels`` is one-hot, ``labels.T @ labels == diag(counts)``,
so the whole thing collapses to a single pass over ``labels``:

    S      = labels.T @ embeded_preds         # [C,D] per-class sum of preds
    counts = column sums of labels            # [C]
    out    = beta * center + gamma * S
             beta  = 1 - 0.5*counts/(counts+1)
             gamma = 0.5/(counts+1)

Sharding: by CLASS, not batch.  Core i owns classes [i*1250, (i+1)*1250): it
streams labels[:, shard] (the same 82 MB/core the batch split would read),
additionally reads all of preds (only 8.4 MB), and produces its [1250, 128]
slice of the output with NO inter-core communication at all -- the batch-split
variant needs a serial ReduceScatter of the [C,D] partials which measured
~85us of un-overlappable tail.

The 655MB ``labels`` tensor is the whole game (target ~358 GB/s/core HBM):
  * it streams through the PE exactly once as the *moving* matmul operand in
    a single fp32r pass (fp32r moving rows cost 1 PE cycle at width>=256,
    same as bf16; the rounding error lands ~2e-4 rel, far inside the 2e-2
    gate);
  * each SBUF partition line packs FOUR consecutive DRAM rows, so every DMA
    descriptor is a 20 KB contiguous read (a [128 rows-on-partitions] layout
    gives 5 KB descriptors, which caps each of the 16 SDMA engines at
    ~21 GB/s -- measured); the four row-parities just become four stationary
    preds tiles per k-group;
  * the class shard is host-padded 1250 -> 1280 so all matmul chunks are
    512/512/256 wide -- fp32r moving runs 4 cycles/row below width 256;
  * label DMAs alternate between the two HWDGE rings (sync + scalar) so
    ring issue costs overlap across consecutive tiles;
  * preds is declared float32r like labels: the HWDGE DMA rounds
    fp32 -> fp32r in flight, which satisfies the BIR verifier's "matmul
    inputs must be rounded" rule with zero extra compute.
Per-partition partial counts accumulate on the DVE (two parities per
instruction; GpSimd measures ~4us per add AND degrades concurrent DVE ops
~2.7x via SBUF-port interference, so it stays idle) and are reduced by one
accumulating PE pass against a ones vector; the [d, csp] S.T accumulator
stays in PSUM for the whole stream and is transposed on-chip for the tail.
"""

import numpy as np

N, C, D = 16384, 10000, 128
NCORES = 8
CS = C // NCORES   # 1250 classes per core
CSP = 1280         # padded shard width (host pads with zero columns)
LR = 0.5
P = 128
R = 4              # DRAM rows interleaved per SBUF partition line
GR = R * P         # rows per k-group (512)
PJ = 16            # preds load chunks (at full size)


def _chunks(width, step=512):
    out = []
    c0 = 0
    while c0 < width:
        out.append((c0, min(step, width - c0)))
        c0 += step
    return out


def build_program(n=N, cs=CS, csp=CSP, d=D):
    """Build the SPMD Bass program (identical on every core)."""
    import concourse.bacc as bacc
    import concourse.mybir as mybir
    import concourse.tile as tile
    from concourse.masks import make_identity

    f32 = mybir.dt.float32
    f32r = mybir.dt.float32r
    mult = mybir.AluOpType.mult
    add = mybir.AluOpType.add

    assert n % GR == 0
    ng = n // GR             # k-groups (32)
    pj = min(PJ, ng)         # preds chunks
    pgc = ng // pj           # k-groups per preds chunk
    assert ng % pj == 0
    nt3 = (cs + P - 1) // P  # class tiles for the final update (10)

    nc = bacc.Bacc(
        "TRN2",
        target_bir_lowering=False,
        debug=False,
        num_devices=NCORES,
    )

    preds = nc.dram_tensor("preds", [n, d], f32r, kind="ExternalInput").ap()
    labels = nc.dram_tensor("labels", [n, csp], f32r, kind="ExternalInput").ap()
    center = nc.dram_tensor("center", [cs, d], f32, kind="ExternalInput").ap()
    out = nc.dram_tensor("out", [cs, d], f32, kind="ExternalOutput").ap()

    with tile.TileContext(nc) as tc:
        with tc.tile_pool(name="const", bufs=1) as const_pool:
            identity = const_pool.tile([P, P], f32, name="identity")
            make_identity(nc, identity[:])
            ones_col = const_pool.tile([P, 1], f32, name="ones_col")
            nc.vector.memset(ones_col[:], 1.0)

            # all of preds as ng*R stationary [K=128, M=d] tiles (one per
            # k-group x row-parity); rounded to f32r by the DMA itself.
            preds_r = const_pool.tile([P, ng * R * d], f32r, name="preds_r")

            # per-partition partial counts, accumulated on DVE.  Two
            # parities are added per instruction ([128, 2*csp]); the column
            # blocks are summed against each other by the final accumulating
            # PE reduce, so no combine pass is needed.
            cnt_sb = const_pool.tile([P, 2 * csp], f32, name="cnt_sb")
            # S.T evacuated from PSUM, and the reduced counts row
            st_sb = const_pool.tile([P, cs], f32, name="st_sb")
            cnt_row = const_pool.tile([1, csp], f32, name="cnt_row")
            # center shard, as nt3 [class, d] tiles (class on partitions)
            ctr_sb = const_pool.tile([P, nt3 * d], f32, name="ctr_sb")

            # ---------------- phase 1: stream labels ----------------
            with (
                tc.tile_pool(name="lab", bufs=5) as lab_pool,
                tc.tile_pool(name="psum1", bufs=1, space="PSUM") as psum1,
            ):
                st_psum = psum1.tile([d, csp], f32, name="st_psum", tag="st",
                                     space="PSUM")

                def load_preds_chunk(j, eng):
                    # one preds chunk (pgc k-groups): partition p takes rows
                    # 4p..4p+3 of each k-group (2 KB contiguous descriptors).
                    # Emission order IS dataflow order in Tile, so chunk j
                    # must be emitted before the first matmul that reads it,
                    # with enough lead: small descriptors get starved by the
                    # 20 KB label packets in the engines' per-packet
                    # round-robin (~14us for a 1 MB chunk under load).
                    c0, c1 = j * pgc * R * d, (j + 1) * pgc * R * d
                    eng.dma_start(
                        out=preds_r[:, c0:c1].rearrange(
                            "p (k r d) -> p k r d", k=pgc, r=R),
                        in_=preds[j * pgc * GR:(j + 1) * pgc * GR, :]
                            .rearrange("(k p r) d -> p k r d", p=P, r=R),
                    )

                # chunk 0 goes at the head of the sync ring, before lab 0:
                # it completes in ~2us so the PE starts ~25us earlier than
                # if it queued behind label traffic.
                load_preds_chunk(0, nc.sync)
                for g in range(ng):
                    lab_g = lab_pool.tile([P, R * csp], f32r, name=f"lab_{g}",
                                          tag="lab")
                    # alternate the two HWDGE rings so ring issue costs and
                    # completion latencies overlap across consecutive tiles;
                    # partition p reads DRAM rows g*512+4p..+3 = one 20 KB
                    # contiguous descriptor.
                    eng = nc.sync if g % 2 == 0 else nc.scalar
                    eng.dma_start(
                        out=lab_g[:],
                        in_=labels[g * GR:(g + 1) * GR, :]
                            .rearrange("(p r) c -> p (r c)", r=R),
                    )
                    for j in range(1, pj):
                        # interleave the remaining preds chunks into the
                        # scalar ring, each >=4 groups before its first
                        # consumer (chunk j feeds k-groups g >= j*pgc)
                        if g == max(0, min(2 * j - 4, j * pgc - 1)):
                            load_preds_chunk(j, nc.scalar)
                    if g == min(5, ng - 1):
                        # center shard, needed only for the tail update
                        for tt in range(nt3):
                            w = min(P, cs - tt * P)
                            nc.scalar.dma_start(
                                out=ctr_sb[0:w, tt * d:tt * d + d],
                                in_=center[tt * P:tt * P + w, :],
                            )
                    for q in range(R):
                        stat = preds_r[:, (g * R + q) * d:(g * R + q + 1) * d]
                        for c0, w in _chunks(csp):
                            nc.tensor.matmul(
                                out=st_psum[:, c0:c0 + w],
                                lhsT=stat,
                                rhs=lab_g[:, q * csp + c0:q * csp + c0 + w],
                                start=(g == 0 and q == 0),
                                stop=(g == ng - 1 and q == R - 1),
                            )
                    for h in range(2):
                        if g == 0 and h == 0:
                            nc.vector.tensor_copy(
                                out=cnt_sb[:],
                                in_=lab_g[:, 0:2 * csp].bitcast(f32),
                            )
                        else:
                            nc.vector.tensor_add(
                                out=cnt_sb[:], in0=cnt_sb[:],
                                in1=lab_g[:, h * 2 * csp:
                                          (h + 1) * 2 * csp].bitcast(f32),
                            )

                # ACT evacuates S.T as soon as the accumulation group stops
                # (emitted before the counts reduce so its semaphore wait
                # does not include those matmuls)
                nc.scalar.copy(out=st_sb[:], in_=st_psum[:, 0:cs])

                # reduce the 256 partial count rows (128 partitions x 2
                # column blocks) with accumulating ones matmuls
                cnt_psum = psum1.tile([P, 3 * 512], f32, name="cnt_psum",
                                      tag="cnt", space="PSUM")
                for h in range(2):
                    for c0, w in _chunks(csp):
                        nc.tensor.matmul(
                            out=cnt_psum[0:1, c0:c0 + w],
                            lhsT=ones_col[:],
                            rhs=cnt_sb[:, h * csp + c0:h * csp + c0 + w],
                            start=(h == 0),
                            stop=(h == 1),
                        )
                nc.scalar.copy(out=cnt_row[:], in_=cnt_psum[0:1, 0:csp])

                # ---------------- tail: update this core's shard ----------
                # counts column per class tile, transposed into the spare
                # columns of the cnt PSUM bank; padded classes have count 0
                # so every lane stays finite
                for tt in range(nt3):
                    nc.tensor.transpose(
                        out=cnt_psum[0:P, 1300 + tt:1301 + tt],
                        in_=cnt_row[0:1, tt * P:(tt + 1) * P],
                        identity=identity[0:1, 0:1],
                    )
                cnt_col = cnt_psum[:, 1300:1300 + nt3]

                with tc.tile_pool(name="p3", bufs=1) as p3:
                    # per-class scalars for all nt3 tiles in one [P, nt3]
                    # batch: den = counts+1, gam = 0.5/den,
                    # bet = 1 - 0.5*counts/den
                    den = p3.tile([P, nt3], f32, name="den")
                    nc.vector.tensor_scalar_add(
                        out=den[:], in0=cnt_col, scalar1=1.0
                    )
                    rec = p3.tile([P, nt3], f32, name="rec")
                    nc.vector.reciprocal(out=rec[:], in_=den[:])
                    gam = p3.tile([P, nt3], f32, name="gam")
                    nc.vector.tensor_scalar_mul(
                        out=gam[:], in0=rec[:], scalar1=0.5
                    )
                    bet = p3.tile([P, nt3], f32, name="bet")
                    nc.vector.tensor_tensor(
                        out=bet[:], in0=cnt_col, in1=rec[:], op=mult
                    )
                    nc.vector.tensor_scalar(
                        out=bet[:], in0=bet[:],
                        scalar1=-0.5, scalar2=1.0, op0=mult, op1=add,
                    )
                    for tt in range(nt3):
                        w = min(P, cs - tt * P)
                        trp = psum1.tile([P, d], f32, name=f"trp_{tt}",
                                         tag="trp", bufs=2, space="PSUM")
                        nc.tensor.transpose(
                            out=trp[0:w, 0:d],
                            in_=st_sb[:, tt * P:tt * P + w],
                            identity=identity[:, 0:d],
                        )
                        o1 = p3.tile([P, d], f32, name=f"o1_{tt}", tag="o1",
                                     bufs=2)
                        nc.vector.tensor_scalar_mul(
                            out=o1[0:w, :], in0=ctr_sb[0:w, tt * d:tt * d + d],
                            scalar1=bet[0:w, tt:tt + 1],
                        )
                        ou = p3.tile([P, d], f32, name=f"ou_{tt}", tag="ou",
                                     bufs=2)
                        nc.vector.scalar_tensor_tensor(
                            out=ou[0:w, :], in0=trp[0:w, 0:d],
                            scalar=gam[0:w, tt:tt + 1],
                            in1=o1[0:w, :], op0=mult, op1=add,
                        )
                        nc.sync.dma_start(
                            out=out[tt * P:tt * P + w, :], in_=ou[0:w, 0:d]
                        )

    nc.compile()
    return nc


_PROGRAM = None
LAST_RESULTS = None  # BassKernelResults from the most recent run (for test.py)


def _get_program():
    global _PROGRAM
    if _PROGRAM is None:
        _PROGRAM = build_program()
    return _PROGRAM


def kernel(embeded_preds, labels, center):
    from concourse.bass_utils import run_bass_kernel_spmd

    global LAST_RESULTS
    preds = np.ascontiguousarray(np.asarray(embeded_preds, dtype=np.float32))
    lab = np.asarray(labels, dtype=np.float32)
    ctr = np.ascontiguousarray(np.asarray(center, dtype=np.float32))
    assert preds.shape == (N, D) and lab.shape == (N, C) and ctr.shape == (C, D)

    nc = _get_program()
    in_maps = []
    for i in range(NCORES):
        shard = np.zeros((N, CSP), np.float32)
        shard[:, :CS] = lab[:, i * CS:(i + 1) * CS]
        in_maps.append({
            "preds": preds,
            "labels": shard,
            "center": np.ascontiguousarray(ctr[i * CS:(i + 1) * CS]),
        })
    res = run_bass_kernel_spmd(nc, in_maps, core_ids=list(range(NCORES)))
    LAST_RESULTS = res
    return np.concatenate([res.results[i]["out"] for i in range(NCORES)], axis=0)
